# revision 1
# baseline (speedup 1.0000x reference)
"""3-layer GAT on 8 Trainium2 NeuronCores (Bass/Tile, SPMD).

Strategy (dst-sharded edge parallelism):
- Pad N to NPAD = 8*128*k nodes; core c owns the contiguous node range
  [c*NPC, (c+1)*NPC) and processes exactly the edges whose dst falls in
  its range (edges sorted by dst on host). Weights replicated.
- Per layer: every core computes the full feature table
  pack = h @ [W | W@al] -> DRAM rows [feat | el] (replicated compute,
  zero communication), then edge phase: for each 128-edge tile,
  feat[src] rows arrive via indirect DMA gather (128 rows/call, one
  row per partition); one-hot matrices built on-device from dst values
  (iota + is_equal) turn segment-sum / per-dst-broadcast into PE
  matmuls accumulated in PSUM per 128-node block. Softmax denominators
  and message sums are divided once per node, post-aggregation
  (exp(e)/sum exp(e) == softmax exactly; no max-subtraction needed at
  these magnitudes).
- Between layers: each core's output block is transposed on the PE and
  AllGathered (transposed layout feeds the next layer's matmul with no
  further transposes).
"""
import numpy as np
from contextlib import ExitStack

import os
import concourse.bass as bass
import concourse.mybir as mybir
import concourse.tile as tile
from concourse.bass_utils import run_bass_kernel_spmd
from concourse.masks import make_identity

try:
    import bass_rust
except ImportError:  # pragma: no cover
    bass_rust = None

F32 = mybir.dt.float32
I32 = mybir.dt.int32
ALU = mybir.AluOpType
ACT = mybir.ActivationFunctionType
P = 128
NC = 8
NEG_SLOPE = 0.2

_ws_ctr = [0]


def _split_waits(nc, limit=1):
    """This container's walrus encodes at most `limit` sem waits per
    instruction. Hoist extras onto same-engine NoOps placed before."""
    for fn in nc.m.functions:
        for bb in fn.blocks:
            insts = bb.instructions
            if not any(
                i.sync_info is not None and len(i.sync_info.on_wait) > limit
                for i in insts
            ):
                continue
            out = []
            for ins in insts:
                si = ins.sync_info
                if si is not None and len(si.on_wait) > limit:
                    waits = list(si.on_wait)
                    extra, keep = waits[:-limit], waits[-limit:]
                    for w in extra:
                        _ws_ctr[0] += 1
                        nop = mybir.InstNoOp(
                            name=f"I-waitsplit-{_ws_ctr[0]}", ins=[], outs=[]
                        )
                        nop.engine = ins.engine
                        nop.sync_info = bass_rust.SyncInfo(on_wait=[w], on_update=[])
                        out.append(nop)
                    ins.sync_info = bass_rust.SyncInfo(
                        on_wait=keep, on_update=list(si.on_update)
                    )
                out.append(ins)
            bb.instructions = out


def _pack_cols(n):  # pad row length to a 64-float multiple for clean strides
    return ((n + 63) // 64) * 64


def _build_program(NPAD, T_B, NB):
    NPC = NPAD // NC
    NT = NB * T_B
    NSEC_TILES = NPC // P          # n-tiles per core-section (== NB)
    GW = 512                        # group width for matmul-phase loads

    nc = bass.Bass(num_devices=NC)

    C0, C1, C2 = 260, 260, 65       # packed cols: feat + el per layer
    E0, E1, E2 = _pack_cols(C0), _pack_cols(C1), _pack_cols(C2)
    F0, F1, F2 = 256, 256, 64       # feat widths
    H0, H1, H2 = 4, 4, 1            # heads

    # ---- DRAM tensors ----
    xT = nc.dram_tensor("xT", [256, NPAD], F32, kind="ExternalInput")
    xTo = nc.dram_tensor("xTo", [256, NPC], F32, kind="ExternalInput")
    idx_h = nc.dram_tensor("idx", [P, NT], I32, kind="ExternalInput")
    dstv_h = nc.dram_tensor("dstv", [P, NT], F32, kind="ExternalInput")
    wag = [
        nc.dram_tensor(f"wag{i}", [2, P, c], F32, kind="ExternalInput")
        for i, c in enumerate((C0, C1, C2))
    ]
    war = [
        nc.dram_tensor(f"war{i}", [2, P, h], F32, kind="ExternalInput")
        for i, h in enumerate((H0, H1, H2))
    ]
    rw2 = nc.dram_tensor("rw2", [2, P, 64], F32, kind="ExternalInput")
    bia = [
        nc.dram_tensor(f"bias{i}", [P, f], F32, kind="ExternalInput")
        for i, f in enumerate((F0, F1, F2))
    ]
    out2 = nc.dram_tensor("out2", [NPC, 64], F32, kind="ExternalOutput")

    pack = [
        nc.dram_tensor(f"pack{i}", [NPAD, e], F32)
        for i, e in enumerate((E0, E1, E2))
    ]
    hown = nc.dram_tensor("hown", [NPC, 256], F32)
    agin = [nc.dram_tensor(f"agin{i}", [256, NPC], F32) for i in range(2)]
    agout = [
        nc.dram_tensor(f"agout{i}", [NC, 256, NPC], F32, addr_space="Shared")
        for i in range(2)
    ]

    with tile.TileContext(nc) as tc, ExitStack() as ctx:
        cst = ctx.enter_context(tc.tile_pool(name="cst", bufs=1))
        ld = ctx.enter_context(tc.tile_pool(name="ld", bufs=6))
        stg = ctx.enter_context(tc.tile_pool(name="stg", bufs=6))
        gp = ctx.enter_context(tc.tile_pool(name="gp", bufs=10))
        ep = ctx.enter_context(tc.tile_pool(name="ep", bufs=8))
        # PSUM: 8 banks/partition total; every tile is padded to one bank.
        mp = ctx.enter_context(tc.tile_pool(name="mp", bufs=1, space="PSUM"))      # mmps -> 1
        m1p = ctx.enter_context(tc.tile_pool(name="m1p", bufs=2, space="PSUM"))    # m1tps -> 2
        sp = ctx.enter_context(tc.tile_pool(name="sp", bufs=2, space="PSUM"))      # ereps -> 2
        rp_ = ctx.enter_context(tc.tile_pool(name="rp", bufs=1, space="PSUM"))     # resps -> 1
        agp = ctx.enter_context(tc.tile_pool(name="agp", bufs=2, space="PSUM"))    # agg -> 2

        # ---- constants ----
        idx_sb = cst.tile([P, NT], I32, tag="idx")
        nc.sync.dma_start(idx_sb[:], idx_h[:])
        dstv_sb = cst.tile([P, NT], F32, tag="dstv")
        nc.sync.dma_start(dstv_sb[:], dstv_h[:])
        iota_i = cst.tile([P, P], I32, tag="ioi")
        nc.gpsimd.iota(iota_i[:], [[1, P]], channel_multiplier=0)
        iota_f = cst.tile([P, P], F32, tag="iof")
        nc.vector.tensor_copy(iota_f[:], iota_i[:])
        ident = cst.tile([P, P], F32, tag="id")
        make_identity(nc, ident[:])
        wag_sb = []
        for i, c in enumerate((C0, C1, C2)):
            t = cst.tile([P, 2, c], F32, tag=f"wag{i}")
            nc.sync.dma_start(t[:, 0, :], wag[i][0])
            nc.sync.dma_start(t[:, 1, :], wag[i][1])
            wag_sb.append(t)
        war_sb = []
        for i, h in enumerate((H0, H1, H2)):
            t = cst.tile([P, 2, h], F32, tag=f"war{i}")
            nc.sync.dma_start(t[:, 0, :], war[i][0])
            nc.sync.dma_start(t[:, 1, :], war[i][1])
            war_sb.append(t)
        rw2_sb = cst.tile([P, 2, 64], F32, tag="rw2")
        nc.sync.dma_start(rw2_sb[:, 0, :], rw2[0])
        nc.sync.dma_start(rw2_sb[:, 1, :], rw2[1])
        bia_sb = []
        for i, f in enumerate((F0, F1, F2)):
            t = cst.tile([P, f], F32, tag=f"bia{i}")
            nc.sync.dma_start(t[:], bia[i][:])
            bia_sb.append(t)
        ero_sb = [
            cst.tile([P, NB * h], F32, tag=f"ero{i}", name=f"ero{i}")
            for i, h in enumerate((H0, H1, H2))
        ]

        def mm_phase(L, ncols, elems, pk):
            """pack rows = h @ wag for all NPAD nodes."""
            wt = wag_sb[L]
            for sec in range(NC):
                off = 0
                while off < NPC:
                    g = min(GW, NPC - off)
                    h0 = ld.tile([P, GW], F32, tag="h0")
                    h1 = ld.tile([P, GW], F32, tag="h1")
                    if L == 0:
                        base = sec * NPC + off
                        nc.sync.dma_start(h0[:, :g], xT[0:P, base:base + g])
                        nc.sync.dma_start(h1[:, :g], xT[P:2 * P, base:base + g])
                    else:
                        src = agout[L - 1]
                        nc.sync.dma_start(h0[:, :g], src[sec, 0:P, off:off + g])
                        nc.sync.dma_start(h1[:, :g], src[sec, P:2 * P, off:off + g])
                    for i in range(g // P):
                        ps = mp.tile([P, ncols], F32, tag="mmps", space="PSUM")
                        nc.tensor.matmul(out=ps[:], lhsT=h0[:, i * P:(i + 1) * P],
                                         rhs=wt[:, 0, :], start=True, stop=False)
                        nc.tensor.matmul(out=ps[:], lhsT=h1[:, i * P:(i + 1) * P],
                                         rhs=wt[:, 1, :], start=False, stop=True)
                        st = stg.tile([P, elems], F32, tag="stg")
                        nc.vector.tensor_copy(st[:, 0:ncols], ps[:])
                        row0 = sec * NPC + off + i * P
                        nc.sync.dma_start(pk[row0:row0 + P, :], st[:])
                    off += g

        def er_own_phase(L, h_src):
            """er for own nodes: (h_ownT chunks).T @ (W@ar)."""
            H = (H0, H1, H2)[L]
            for b in range(NB):
                c0 = ld.tile([P, P], F32, tag="ec0")
                c1 = ld.tile([P, P], F32, tag="ec1")
                if h_src is None:
                    nc.sync.dma_start(c0[:], xTo[0:P, b * P:(b + 1) * P])
                    nc.sync.dma_start(c1[:], xTo[P:2 * P, b * P:(b + 1) * P])
                else:
                    nc.sync.dma_start(c0[:], h_src[0:P, b * P:(b + 1) * P])
                    nc.sync.dma_start(c1[:], h_src[P:2 * P, b * P:(b + 1) * P])
                ps = sp.tile([P, max(H0, H1, H2)], F32, tag="ereps", name="ereps", space="PSUM")[:, 0:H]
                nc.tensor.matmul(out=ps[:], lhsT=c0[:], rhs=war_sb[L][:, 0, :],
                                 start=True, stop=False)
                nc.tensor.matmul(out=ps[:], lhsT=c1[:], rhs=war_sb[L][:, 1, :],
                                 start=False, stop=True)
                nc.vector.tensor_copy(ero_sb[L][:, b * H:(b + 1) * H], ps[:])

        def edge_phase(L, pk, elems, F, H, write_sinks):
            bias_t = bia_sb[L]
            for b in range(NB):
                agf = agp.tile([P, 272], F32, tag="agg", name="agg", space="PSUM")
                agg = agf[:, 0:F]
                den = agf[:, F:F + H]
                for t in range(T_B):
                    tt = b * T_B + t
                    g = gp.tile([P, elems], F32, tag="g")
                    if os.environ.get("GAT_DIAG_STREAM"):
                        nc.sync.dma_start(g[:], pk[(tt % (NPAD // P)) * P:(tt % (NPAD // P)) * P + P, :])
                    else:
                        nc.gpsimd.indirect_dma_start(
                            out=g[:], out_offset=None, in_=pk[:],
                            in_offset=bass.IndirectOffsetOnAxis(
                                ap=idx_sb[:, tt:tt + 1], axis=0))
                    m1 = ep.tile([P, P], F32, tag="m1")
                    nc.vector.tensor_tensor(
                        out=m1[:], in0=dstv_sb[:, tt:tt + 1].to_broadcast([P, P]),
                        in1=iota_f[:], op=ALU.is_equal)
                    m1t_ps = m1p.tile([P, P], F32, tag="m1tps", space="PSUM")
                    nc.tensor.transpose(out=m1t_ps[:], in_=m1[:], identity=ident[:])
                    m1t = ep.tile([P, P], F32, tag="m1t")
                    nc.vector.tensor_copy(m1t[:], m1t_ps[:])
                    ere = sp.tile([P, max(H0, H1, H2)], F32, tag="ereps", name="ereps", space="PSUM")[:, 0:H]
                    nc.tensor.matmul(out=ere[:], lhsT=m1t[:],
                                     rhs=ero_sb[L][:, b * H:(b + 1) * H],
                                     start=True, stop=True)
                    e_sb = ep.tile([P, H], F32, tag="e")
                    nc.vector.tensor_add(e_sb[:], g[:, F:F + H], ere[:])
                    nc.vector.scalar_tensor_tensor(
                        out=e_sb[:], in0=e_sb[:], scalar=NEG_SLOPE, in1=e_sb[:],
                        op0=ALU.mult, op1=ALU.max)
                    sc = ep.tile([P, F + H], F32, tag="sc")
                    ee = sc[:, F:F + H]
                    nc.scalar.activation(ee, e_sb[:], ACT.Exp)
                    nc.vector.tensor_tensor(
                        out=sc[:, 0:F].rearrange("p (h d) -> p h d", h=H),
                        in0=g[:, 0:F].rearrange("p (h d) -> p h d", h=H),
                        in1=ee.to_broadcast([P, H, F // H]), op=ALU.mult)
                    nc.tensor.matmul(out=agf[:, 0:F + H], lhsT=m1[:], rhs=sc[:],
                                     start=(t == 0), stop=(t == T_B - 1))
                # epilogue
                den_c = ep.tile([P, H], F32, tag="denc")
                nc.vector.tensor_scalar_max(den_c[:], den[:], 1e-30)
                rec = ep.tile([P, H], F32, tag="rec")
                nc.vector.reciprocal(rec[:], den_c[:])
                o = ep.tile([P, F], F32, tag="o")
                nc.vector.tensor_tensor(
                    out=o[:].rearrange("p (h d) -> p h d", h=H),
                    in0=agg[:].rearrange("p (h d) -> p h d", h=H),
                    in1=rec[:].to_broadcast([P, H, F // H]), op=ALU.mult)
                write_sinks(b, o)

        def sink_l0(b, o):
            nc.vector.tensor_add(o[:], o[:], bia_sb[0][:])
            _elu(o)
            nc.sync.dma_start(hown[b * P:(b + 1) * P, :], o[:])
            _write_agin(agin[0], b, o)

        def sink_l1(b, o):
            hb = ld.tile([P, 256], F32, tag="hb")
            nc.sync.dma_start(hb[:], hown[b * P:(b + 1) * P, :])
            nc.vector.tensor_add(o[:], o[:], hb[:])
            nc.vector.tensor_add(o[:], o[:], bia_sb[1][:])
            _elu(o)
            _write_agin(agin[1], b, o)

        def sink_l2(b, o):
            r0 = ld.tile([P, P], F32, tag="r0")
            r1 = ld.tile([P, P], F32, tag="r1")
            nc.sync.dma_start(r0[:], agin[1][0:P, b * P:(b + 1) * P])
            nc.sync.dma_start(r1[:], agin[1][P:2 * P, b * P:(b + 1) * P])
            rp = rp_.tile([P, 64], F32, tag="resps", space="PSUM")
            nc.tensor.matmul(out=rp[:], lhsT=r0[:], rhs=rw2_sb[:, 0, :],
                             start=True, stop=False)
            nc.tensor.matmul(out=rp[:], lhsT=r1[:], rhs=rw2_sb[:, 1, :],
                             start=False, stop=True)
            nc.vector.tensor_add(o[:], o[:], rp[:])
            nc.vector.tensor_add(o[:], o[:], bia_sb[2][:])
            nc.sync.dma_start(out2[b * P:(b + 1) * P, :], o[:])

        def _elu(o):
            mx = ep.tile([P, 256], F32, tag="mx")
            nc.vector.tensor_scalar_max(mx[:], o[:], 0.0)
            mn = ep.tile([P, 256], F32, tag="mn")
            nc.vector.tensor_scalar_min(mn[:], o[:], 0.0)
            exn = ep.tile([P, 256], F32, tag="exn")
            nc.scalar.activation(exn[:], mn[:], ACT.Exp)
            nc.vector.scalar_tensor_tensor(
                out=o[:], in0=exn[:], scalar=-1.0, in1=mx[:],
                op0=ALU.add, op1=ALU.add)

        def _write_agin(ag, b, o):
            t1 = m1p.tile([P, P], F32, tag="m1tps", space="PSUM")
            nc.tensor.transpose(out=t1[:], in_=o[:, 0:P], identity=ident[:])
            ot1 = ep.tile([P, P], F32, tag="ot1")
            nc.vector.tensor_copy(ot1[:], t1[:])
            nc.sync.dma_start(ag[0:P, b * P:(b + 1) * P], ot1[:])
            t2 = m1p.tile([P, P], F32, tag="m1tps", space="PSUM")
            nc.tensor.transpose(out=t2[:], in_=o[:, P:2 * P], identity=ident[:])
            ot2 = ep.tile([P, P], F32, tag="ot2")
            nc.vector.tensor_copy(ot2[:], t2[:])
            nc.sync.dma_start(ag[P:2 * P, b * P:(b + 1) * P], ot2[:])

        def allgather(i):
            tc.strict_bb_all_engine_barrier()
            nc.gpsimd.collective_compute(
                "AllGather", ALU.bypass, replica_groups=[list(range(NC))],
                ins=[agin[i][:]], outs=[agout[i][:]])
            tc.strict_bb_all_engine_barrier()

        # ---- layer 0 ----
        mm_phase(0, C0, E0, pack[0])
        er_own_phase(0, None)
        tc.strict_bb_all_engine_barrier()
        edge_phase(0, pack[0], E0, F0, H0, sink_l0)
        allgather(0)
        # ---- layer 1 ----
        mm_phase(1, C1, E1, pack[1])
        er_own_phase(1, agin[0])
        tc.strict_bb_all_engine_barrier()
        edge_phase(1, pack[1], E1, F1, H1, sink_l1)
        allgather(1)
        # ---- layer 2 ----
        mm_phase(2, C2, E2, pack[2])
        er_own_phase(2, agin[1])
        tc.strict_bb_all_engine_barrier()
        edge_phase(2, pack[2], E2, F2, H2, sink_l2)

    _split_waits(nc, limit=1)
    return nc


def prepare(**inputs):
    x = np.asarray(inputs["x"], dtype=np.float32)
    src = np.asarray(inputs["src"], dtype=np.int64)
    dst = np.asarray(inputs["dst"], dtype=np.int64)
    N, IND = x.shape
    NPAD = ((N + NC * P - 1) // (NC * P)) * (NC * P)
    NPC = NPAD // NC
    NB = NPC // P

    # ---- host-side graph preprocessing (sharding) ----
    core = dst // NPC
    blk = (dst % NPC) // P
    dv = (dst % P).astype(np.float32)
    order = np.lexsort((src, blk, core))
    src_s, core_s, blk_s, dv_s = src[order], core[order], blk[order], dv[order]
    # per (core, block) counts
    counts = np.zeros((NC, NB), dtype=np.int64)
    np.add.at(counts, (core_s, blk_s), 1)
    T_B = int(np.max((counts + P - 1) // P))
    NT = NB * T_B
    idx_all = np.zeros((NC, NT * P), dtype=np.int32)          # pad idx -> row 0
    dvv_all = np.full((NC, NT * P), 999.0, dtype=np.float32)  # pad dstv OOR
    bounds = np.zeros((NC, NB + 1), dtype=np.int64)
    for c in range(NC):
        m = core_s == c
        bc = np.concatenate([[0], np.cumsum(counts[c])])
        bounds[c] = bc
        sc, bs, dc = src_s[m], blk_s[m], dv_s[m]
        for b in range(NB):
            seg = slice(bc[b], bc[b + 1])
            n = bc[b + 1] - bc[b]
            base = b * T_B * P
            idx_all[c, base:base + n] = sc[seg]
            dvv_all[c, base:base + n] = dc[seg]
    # wrap position i -> (partition i%128, col i//128)
    idx_maps = idx_all.reshape(NC, NT, P).transpose(0, 2, 1)   # [NC, P, NT]
    dvv_maps = dvv_all.reshape(NC, NT, P).transpose(0, 2, 1)

    # ---- weights prep ----
    def aug(W, al):
        H, D = al.shape
        alc = np.stack([W[:, h * D:(h + 1) * D] @ al[h] for h in range(H)], axis=1)
        return np.concatenate([W, alc], axis=1).astype(np.float32)

    def arc(W, ar):
        H, D = ar.shape
        return np.stack(
            [W[:, h * D:(h + 1) * D] @ ar[h] for h in range(H)], axis=1
        ).astype(np.float32)

    W0, al0, ar0 = inputs["W0"], inputs["al0"], inputs["ar0"]
    W1, al1, ar1 = inputs["W1"], inputs["al1"], inputs["ar1"]
    W2, al2, ar2 = inputs["W2"], inputs["al2"], inputs["ar2"]
    wag0, war0 = aug(W0, al0), arc(W0, ar0)
    wag1, war1 = aug(W1, al1), arc(W1, ar1)
    wag2, war2 = aug(W2, al2), arc(W2, ar2)
    b0 = np.asarray(inputs["b0"], np.float32)
    b1 = np.asarray(inputs["b1"], np.float32)
    b2 = np.asarray(inputs["b2"], np.float32)
    rw2 = np.asarray(inputs["res_w2"], np.float32)

    xpad = np.zeros((NPAD, IND), np.float32)
    xpad[:N] = x
    xT = np.ascontiguousarray(xpad.T)                          # [256, NPAD]

    nc = _build_program(NPAD, T_B, NB)

    def chunks2(W):  # [256, C] -> [2, 128, C]
        return np.stack([W[0:P], W[P:2 * P]]).astype(np.float32)

    in_maps = []
    for c in range(NC):
        in_maps.append({
            "xT": xT,
            "xTo": np.ascontiguousarray(xT[:, c * NPC:(c + 1) * NPC]),
            "idx": np.ascontiguousarray(idx_maps[c]),
            "dstv": np.ascontiguousarray(dvv_maps[c]),
            "wag0": chunks2(wag0), "wag1": chunks2(wag1), "wag2": chunks2(wag2),
            "war0": chunks2(war0), "war1": chunks2(war1), "war2": chunks2(war2),
            "rw2": chunks2(rw2),
            "bias0": np.tile(b0[None, :], (P, 1)).astype(np.float32),
            "bias1": np.tile(b1[None, :], (P, 1)).astype(np.float32),
            "bias2": np.tile(b2[None, :], (P, 1)).astype(np.float32),
        })

    return nc, in_maps, N


def kernel(**inputs):
    nc, in_maps, N = prepare(**inputs)
    import time as _time
    _t0 = _time.time()
    res = run_bass_kernel_spmd(nc, in_maps, list(range(NC)))
    global LAST_EXEC_WALL
    LAST_EXEC_WALL = _time.time() - _t0
    out = np.concatenate([res.results[c]["out2"] for c in range(NC)], axis=0)
    return out[:N].astype(np.float32)



# revision 4
# speedup vs baseline: 7.7285x; 7.7285x over previous
"""3-layer GAT on 8 Trainium2 NeuronCores (Bass/Tile, SPMD).

Strategy (dst-sharded edge parallelism, shard-computed pack tables):
- Pad N to NPAD = 8*128*k nodes; core c owns the contiguous node range
  [c*NPC, (c+1)*NPC) and processes exactly the edges whose dst falls in
  its range (edges sorted by dst on host). Weights replicated.
- Host->device traffic is minimized (the axon tunnel runs at ~40 MB/s,
  so it dominates wall time): each core receives ONLY its own slice of
  x (transposed, 6.4 MB) plus edge index maps; the per-node feature
  table pack = h @ [W | W@al | W@ar] is computed for OWN nodes only and
  AllGathered on-device (NeuronLink), never shipped from host.
- Per layer: mm phase computes pack rows [feat | el | er] for own
  nodes; AllGather -> full table; edge phase: for each 128-edge tile,
  pack[src] rows arrive via indirect DMA gather and er[dst] via a
  second narrow indirect gather; one-hot matrices built on-device from
  dst%128 values (iota + is_equal) turn segment-sum into PE matmuls
  accumulated in PSUM per 128-node block. Softmax denominators are
  divided once per node post-aggregation (exp(e)/sum exp(e) == softmax
  exactly; no max-subtraction needed at these magnitudes).
- Edge-phase element-wise work is batched K tiles at a time with
  broadcast access patterns to cut instruction count.
- The built Bass program and the XLA/NEFF executable are cached across
  kernel() calls (jax persistent compilation cache), so warm calls skip
  the multi-second walrus compile.
"""
import os
import numpy as np
from contextlib import ExitStack

import jax

try:
    jax.config.update("jax_compilation_cache_dir",
                      os.path.expanduser("~/.cache/jax_comp_cache"))
    jax.config.update("jax_persistent_cache_min_compile_time_secs", 0.5)
    jax.config.update("jax_persistent_cache_min_entry_size_bytes", 0)
except Exception:
    pass

import concourse.bass as bass
import concourse.mybir as mybir
import concourse.tile as tile
from concourse.bass_utils import run_bass_kernel_spmd
from concourse.masks import make_identity

try:
    import bass_rust
except ImportError:  # pragma: no cover
    bass_rust = None

F32 = mybir.dt.float32
I32 = mybir.dt.int32
ALU = mybir.AluOpType
ACT = mybir.ActivationFunctionType
P = 128
NC = 8
NEG_SLOPE = 0.2
K_GRP = 6

_ws_ctr = [0]


def _split_waits(nc, limit=1):
    """This container's walrus encodes at most `limit` sem waits per
    instruction. Hoist extras onto same-engine NoOps placed before."""
    for fn in nc.m.functions:
        for bb in fn.blocks:
            insts = bb.instructions
            if not any(
                i.sync_info is not None and len(i.sync_info.on_wait) > limit
                for i in insts
            ):
                continue
            out = []
            for ins in insts:
                si = ins.sync_info
                if si is not None and len(si.on_wait) > limit:
                    waits = list(si.on_wait)
                    extra, keep = waits[:-limit], waits[-limit:]
                    for w in extra:
                        _ws_ctr[0] += 1
                        nop = mybir.InstNoOp(
                            name=f"I-waitsplit-{_ws_ctr[0]}", ins=[], outs=[]
                        )
                        nop.engine = ins.engine
                        nop.sync_info = bass_rust.SyncInfo(on_wait=[w], on_update=[])
                        out.append(nop)
                    ins.sync_info = bass_rust.SyncInfo(
                        on_wait=keep, on_update=list(si.on_update)
                    )
                out.append(ins)
            bb.instructions = out


def _pack_cols(n):  # pad row length to a 64-float multiple for clean strides
    return ((n + 63) // 64) * 64


def _build_program(NPAD, T_B, NB):
    NPC = NPAD // NC
    NT = NB * T_B

    nc = bass.Bass(num_devices=NC)

    F_ = (256, 256, 64)             # feature width per layer
    H_ = (4, 4, 1)                  # heads per layer
    C_ = tuple(f + 2 * h for f, h in zip(F_, H_))   # feat + el + er
    E_ = tuple(_pack_cols(c) for c in C_)           # padded pack row

    # ---- DRAM tensors ----
    xTo = nc.dram_tensor("xTo", [256, NPC], F32, kind="ExternalInput")
    idx_h = nc.dram_tensor("idx", [P, NT], I32, kind="ExternalInput")
    dgi_h = nc.dram_tensor("dgi", [P, NT], I32, kind="ExternalInput")
    dstv_h = nc.dram_tensor("dstv", [P, NT], F32, kind="ExternalInput")
    wag = [
        nc.dram_tensor(f"wag{i}", [2, P, c], F32, kind="ExternalInput")
        for i, c in enumerate(C_)
    ]
    rw2 = nc.dram_tensor("rw2", [2, P, 64], F32, kind="ExternalInput")
    bia = [
        nc.dram_tensor(f"bias{i}", [P, f], F32, kind="ExternalInput")
        for i, f in enumerate(F_)
    ]
    out2 = nc.dram_tensor("out2", [NPC, 64], F32, kind="ExternalOutput")

    pko = [
        nc.dram_tensor(f"pko{i}", [NPC, e], F32) for i, e in enumerate(E_)
    ]
    pka = [
        nc.dram_tensor(f"pka{i}", [NPAD, e], F32, addr_space="Shared")
        for i, e in enumerate(E_)
    ]
    hown = nc.dram_tensor("hown", [NPC, 256], F32)
    agin = [nc.dram_tensor(f"agin{i}", [256, NPC], F32) for i in range(2)]

    with tile.TileContext(nc) as tc, ExitStack() as ctx:
        cst = ctx.enter_context(tc.tile_pool(name="cst", bufs=1))
        ld = ctx.enter_context(tc.tile_pool(name="ld", bufs=4))
        stg = ctx.enter_context(tc.tile_pool(name="stg", bufs=4))
        gp = ctx.enter_context(tc.tile_pool(name="gp", bufs=3))
        ep = ctx.enter_context(tc.tile_pool(name="ep", bufs=3))
        sm = ctx.enter_context(tc.tile_pool(name="sm", bufs=4))
        # PSUM: 8 banks/partition total; every tile is padded to one bank.
        mp = ctx.enter_context(tc.tile_pool(name="mp", bufs=2, space="PSUM"))
        m1p = ctx.enter_context(tc.tile_pool(name="m1p", bufs=2, space="PSUM"))
        rp_ = ctx.enter_context(tc.tile_pool(name="rp", bufs=1, space="PSUM"))
        agp = ctx.enter_context(tc.tile_pool(name="agp", bufs=2, space="PSUM"))

        # ---- constants ----
        idx_sb = cst.tile([P, NT], I32, tag="idx")
        nc.sync.dma_start(idx_sb[:], idx_h[:])
        dgi_sb = cst.tile([P, NT], I32, tag="dgi")
        nc.sync.dma_start(dgi_sb[:], dgi_h[:])
        dstv_sb = cst.tile([P, NT], F32, tag="dstv")
        nc.sync.dma_start(dstv_sb[:], dstv_h[:])
        iota_i = cst.tile([P, P], I32, tag="ioi")
        nc.gpsimd.iota(iota_i[:], [[1, P]], channel_multiplier=0)
        iota_f = cst.tile([P, P], F32, tag="iof")
        nc.vector.tensor_copy(iota_f[:], iota_i[:])
        ident = cst.tile([P, P], F32, tag="id")
        make_identity(nc, ident[:])
        wag_sb = []
        for i, c in enumerate(C_):
            t = cst.tile([P, 2, c], F32, tag=f"wag{i}")
            nc.sync.dma_start(t[:, 0, :], wag[i][0])
            nc.sync.dma_start(t[:, 1, :], wag[i][1])
            wag_sb.append(t)
        rw2_sb = cst.tile([P, 2, 64], F32, tag="rw2")
        nc.sync.dma_start(rw2_sb[:, 0, :], rw2[0])
        nc.sync.dma_start(rw2_sb[:, 1, :], rw2[1])
        bia_sb = []
        for i, f in enumerate(F_):
            t = cst.tile([P, f], F32, tag=f"bia{i}")
            nc.sync.dma_start(t[:], bia[i][:])
            bia_sb.append(t)

        def mm_phase(L, h_src):
            """pack rows = h @ [W | W@al | W@ar] for OWN nodes only."""
            C, E = C_[L], E_[L]
            wt = wag_sb[L]
            for b in range(NB):
                c0 = ld.tile([P, P], F32, tag="c0")
                c1 = ld.tile([P, P], F32, tag="c1")
                nc.sync.dma_start(c0[:], h_src[0:P, b * P:(b + 1) * P])
                nc.sync.dma_start(c1[:], h_src[P:2 * P, b * P:(b + 1) * P])
                ps = mp.tile([P, max(C_)], F32, tag="mmps", name="mmps",
                             space="PSUM")[:, 0:C]
                nc.tensor.matmul(out=ps[:], lhsT=c0[:], rhs=wt[:, 0, :],
                                 start=True, stop=False)
                nc.tensor.matmul(out=ps[:], lhsT=c1[:], rhs=wt[:, 1, :],
                                 start=False, stop=True)
                st = stg.tile([P, max(E_)], F32, tag="stg")
                nc.vector.tensor_copy(st[:, 0:C], ps[:])
                nc.sync.dma_start(pko[L][b * P:(b + 1) * P, :], st[:, 0:E])

        def edge_phase(L, write_sinks):
            F, H, E = F_[L], H_[L], E_[L]
            pk = pka[L]
            for b in range(NB):
                agf = agp.tile([P, 272], F32, tag="agg", name="agg", space="PSUM")
                agg = agf[:, 0:F]
                den = agf[:, F:F + H]
                for t0 in range(0, T_B, K_GRP):
                    k = min(K_GRP, T_B - t0)
                    tt0 = b * T_B + t0
                    gw = gp.tile([P, K_GRP, E], F32, tag="gw")
                    ge = gp.tile([P, K_GRP, H], F32, tag="ge")
                    m1w = ep.tile([P, K_GRP, P], F32, tag="m1w")
                    for j in range(k):
                        nc.gpsimd.indirect_dma_start(
                            out=gw[:, j, :], out_offset=None, in_=pk[:],
                            in_offset=bass.IndirectOffsetOnAxis(
                                ap=idx_sb[:, tt0 + j:tt0 + j + 1], axis=0))
                        nc.gpsimd.indirect_dma_start(
                            out=ge[:, j, :], out_offset=None, in_=pk[:],
                            element_offset=F + H,
                            in_offset=bass.IndirectOffsetOnAxis(
                                ap=dgi_sb[:, tt0 + j:tt0 + j + 1], axis=0))
                    # one-hot of dst%128 for k tiles in one op
                    nc.vector.tensor_tensor(
                        out=m1w[:, 0:k, :],
                        in0=dstv_sb[:, tt0:tt0 + k].unsqueeze(2)
                            .to_broadcast([P, k, P]),
                        in1=iota_f[:].unsqueeze(1).to_broadcast([P, k, P]),
                        op=ALU.is_equal)
                    ew = ep.tile([P, K_GRP, H], F32, tag="ew")
                    nc.vector.tensor_add(ew[:, 0:k, :], gw[:, 0:k, F:F + H],
                                         ge[:, 0:k, :])
                    nc.vector.scalar_tensor_tensor(
                        out=ew[:, 0:k, :], in0=ew[:, 0:k, :], scalar=NEG_SLOPE,
                        in1=ew[:, 0:k, :], op0=ALU.mult, op1=ALU.max)
                    sc = ep.tile([P, K_GRP, F + H], F32, tag="sc")
                    nc.scalar.activation(sc[:, 0:k, F:F + H], ew[:, 0:k, :],
                                         ACT.Exp)
                    nc.vector.tensor_tensor(
                        out=sc[:, 0:k, 0:F].rearrange("p k (h d) -> p k h d", h=H),
                        in0=gw[:, 0:k, 0:F].rearrange("p k (h d) -> p k h d", h=H),
                        in1=sc[:, 0:k, F:F + H].unsqueeze(3)
                            .to_broadcast([P, k, H, F // H]),
                        op=ALU.mult)
                    for j in range(k):
                        nc.tensor.matmul(out=agf[:, 0:F + H], lhsT=m1w[:, j, :],
                                         rhs=sc[:, j, :],
                                         start=(t0 + j == 0),
                                         stop=(t0 + j == T_B - 1))
                # epilogue
                den_c = ep.tile([P, H], F32, tag="denc")
                nc.vector.tensor_scalar_max(den_c[:], den[:], 1e-30)
                rec = ep.tile([P, H], F32, tag="rec")
                nc.vector.reciprocal(rec[:], den_c[:])
                o = ep.tile([P, F], F32, tag="o")
                nc.vector.tensor_tensor(
                    out=o[:].rearrange("p (h d) -> p h d", h=H),
                    in0=agg[:].rearrange("p (h d) -> p h d", h=H),
                    in1=rec[:].to_broadcast([P, H, F // H]), op=ALU.mult)
                write_sinks(b, o)

        def sink_l0(b, o):
            nc.vector.tensor_add(o[:], o[:], bia_sb[0][:])
            _elu(o)
            nc.sync.dma_start(hown[b * P:(b + 1) * P, :], o[:])
            _write_agin(agin[0], b, o)

        def sink_l1(b, o):
            hb = ld.tile([P, 256], F32, tag="hb")
            nc.sync.dma_start(hb[:], hown[b * P:(b + 1) * P, :])
            nc.vector.tensor_add(o[:], o[:], hb[:])
            nc.vector.tensor_add(o[:], o[:], bia_sb[1][:])
            _elu(o)
            _write_agin(agin[1], b, o)

        def sink_l2(b, o):
            r0 = ld.tile([P, P], F32, tag="r0")
            r1 = ld.tile([P, P], F32, tag="r1")
            nc.sync.dma_start(r0[:], agin[1][0:P, b * P:(b + 1) * P])
            nc.sync.dma_start(r1[:], agin[1][P:2 * P, b * P:(b + 1) * P])
            rp = rp_.tile([P, 64], F32, tag="resps", space="PSUM")
            nc.tensor.matmul(out=rp[:], lhsT=r0[:], rhs=rw2_sb[:, 0, :],
                             start=True, stop=False)
            nc.tensor.matmul(out=rp[:], lhsT=r1[:], rhs=rw2_sb[:, 1, :],
                             start=False, stop=True)
            nc.vector.tensor_add(o[:], o[:], rp[:])
            nc.vector.tensor_add(o[:], o[:], bia_sb[2][:])
            nc.sync.dma_start(out2[b * P:(b + 1) * P, :], o[:])

        def _elu(o):
            mx = sm.tile([P, 256], F32, tag="mx")
            nc.vector.tensor_scalar_max(mx[:], o[:], 0.0)
            mn = sm.tile([P, 256], F32, tag="mn")
            nc.vector.tensor_scalar_min(mn[:], o[:], 0.0)
            exn = sm.tile([P, 256], F32, tag="exn")
            nc.scalar.activation(exn[:], mn[:], ACT.Exp)
            nc.vector.scalar_tensor_tensor(
                out=o[:], in0=exn[:], scalar=-1.0, in1=mx[:],
                op0=ALU.add, op1=ALU.add)

        def _write_agin(ag, b, o):
            t1 = m1p.tile([P, P], F32, tag="m1tps", space="PSUM")
            nc.tensor.transpose(out=t1[:], in_=o[:, 0:P], identity=ident[:])
            ot1 = sm.tile([P, P], F32, tag="ot1")
            nc.vector.tensor_copy(ot1[:], t1[:])
            nc.sync.dma_start(ag[0:P, b * P:(b + 1) * P], ot1[:])
            t2 = m1p.tile([P, P], F32, tag="m1tps", space="PSUM")
            nc.tensor.transpose(out=t2[:], in_=o[:, P:2 * P], identity=ident[:])
            ot2 = sm.tile([P, P], F32, tag="ot2")
            nc.vector.tensor_copy(ot2[:], t2[:])
            nc.sync.dma_start(ag[P:2 * P, b * P:(b + 1) * P], ot2[:])

        def allgather(L):
            tc.strict_bb_all_engine_barrier()
            nc.gpsimd.collective_compute(
                "AllGather", ALU.bypass, replica_groups=[list(range(NC))],
                ins=[pko[L][:]], outs=[pka[L][:]])
            tc.strict_bb_all_engine_barrier()

        # ---- layer 0 ----
        mm_phase(0, xTo)
        allgather(0)
        edge_phase(0, sink_l0)
        tc.strict_bb_all_engine_barrier()
        # ---- layer 1 ----
        mm_phase(1, agin[0])
        allgather(1)
        edge_phase(1, sink_l1)
        tc.strict_bb_all_engine_barrier()
        # ---- layer 2 ----
        mm_phase(2, agin[1])
        allgather(2)
        edge_phase(2, sink_l2)

    _split_waits(nc, limit=1)
    return nc


_PROG_CACHE = {}


def prepare(**inputs):
    x = np.asarray(inputs["x"], dtype=np.float32)
    src = np.asarray(inputs["src"], dtype=np.int64)
    dst = np.asarray(inputs["dst"], dtype=np.int64)
    N, IND = x.shape
    NPAD = ((N + NC * P - 1) // (NC * P)) * (NC * P)
    NPC = NPAD // NC
    NB = NPC // P

    # ---- host-side graph preprocessing (sharding) ----
    core = dst // NPC
    blk = (dst % NPC) // P
    dv = (dst % P).astype(np.float32)
    order = np.lexsort((src, blk, core))
    src_s, dst_s, core_s, blk_s, dv_s = (
        src[order], dst[order], core[order], blk[order], dv[order])
    # per (core, block) counts
    counts = np.zeros((NC, NB), dtype=np.int64)
    np.add.at(counts, (core_s, blk_s), 1)
    T_B = int(np.max((counts + P - 1) // P))
    NT = NB * T_B
    idx_all = np.zeros((NC, NT * P), dtype=np.int32)          # pad idx -> row 0
    dgi_all = np.zeros((NC, NT * P), dtype=np.int32)          # pad dgi -> row 0
    dvv_all = np.full((NC, NT * P), 999.0, dtype=np.float32)  # pad dstv OOR
    for c in range(NC):
        m = core_s == c
        bc = np.concatenate([[0], np.cumsum(counts[c])])
        sc_, dc_, bs_, dvv_ = src_s[m], dst_s[m], blk_s[m], dv_s[m]
        for b in range(NB):
            seg = slice(bc[b], bc[b + 1])
            n = bc[b + 1] - bc[b]
            base = b * T_B * P
            idx_all[c, base:base + n] = sc_[seg]
            dgi_all[c, base:base + n] = dc_[seg]
            dvv_all[c, base:base + n] = dvv_[seg]
    # wrap position i -> (partition i%128, col i//128)
    idx_maps = idx_all.reshape(NC, NT, P).transpose(0, 2, 1)   # [NC, P, NT]
    dgi_maps = dgi_all.reshape(NC, NT, P).transpose(0, 2, 1)
    dvv_maps = dvv_all.reshape(NC, NT, P).transpose(0, 2, 1)

    # ---- weights prep: wag = [W | W@al | W@ar] ----
    def aug(W, al, ar):
        H, D = al.shape
        alc = np.stack([W[:, h * D:(h + 1) * D] @ al[h] for h in range(H)], axis=1)
        arc = np.stack([W[:, h * D:(h + 1) * D] @ ar[h] for h in range(H)], axis=1)
        return np.concatenate([W, alc, arc], axis=1).astype(np.float32)

    wag0 = aug(inputs["W0"], inputs["al0"], inputs["ar0"])
    wag1 = aug(inputs["W1"], inputs["al1"], inputs["ar1"])
    wag2 = aug(inputs["W2"], inputs["al2"], inputs["ar2"])
    b0 = np.asarray(inputs["b0"], np.float32)
    b1 = np.asarray(inputs["b1"], np.float32)
    b2 = np.asarray(inputs["b2"], np.float32)
    rw2 = np.asarray(inputs["res_w2"], np.float32)

    xpad = np.zeros((NPAD, IND), np.float32)
    xpad[:N] = x

    key = (NPAD, T_B, NB)
    if key not in _PROG_CACHE:
        _PROG_CACHE[key] = _build_program(NPAD, T_B, NB)
    nc = _PROG_CACHE[key]

    def chunks2(W):  # [256, C] -> [2, 128, C]
        return np.stack([W[0:P], W[P:2 * P]]).astype(np.float32)

    in_maps = []
    for c in range(NC):
        in_maps.append({
            "xTo": np.ascontiguousarray(xpad[c * NPC:(c + 1) * NPC].T),
            "idx": np.ascontiguousarray(idx_maps[c]),
            "dgi": np.ascontiguousarray(dgi_maps[c]),
            "dstv": np.ascontiguousarray(dvv_maps[c]),
            "wag0": chunks2(wag0), "wag1": chunks2(wag1), "wag2": chunks2(wag2),
            "rw2": chunks2(rw2),
            "bias0": np.tile(b0[None, :], (P, 1)).astype(np.float32),
            "bias1": np.tile(b1[None, :], (P, 1)).astype(np.float32),
            "bias2": np.tile(b2[None, :], (P, 1)).astype(np.float32),
        })

    return nc, in_maps, N


def kernel(**inputs):
    nc, in_maps, N = prepare(**inputs)
    import time as _time
    _t0 = _time.time()
    res = run_bass_kernel_spmd(nc, in_maps, list(range(NC)))
    global LAST_EXEC_WALL
    LAST_EXEC_WALL = _time.time() - _t0
    out = np.concatenate([res.results[c]["out2"] for c in range(NC)], axis=0)
    return out[:N].astype(np.float32)


# revision 6
# speedup vs baseline: 10.7427x; 1.3900x over previous
"""3-layer GAT on 8 Trainium2 NeuronCores (Bass/Tile, SPMD).

Strategy (dst-sharded edge parallelism, shard-computed pack tables):
- Pad N to NPAD = 8*128*k nodes; core c owns the contiguous node range
  [c*NPC, (c+1)*NPC) and processes exactly the edges whose dst falls in
  its range (edges sorted by dst on host). Weights replicated.
- Host->device traffic is minimized (the axon tunnel runs at ~40 MB/s,
  so it dominates wall time): each core receives ONLY its own slice of
  x (transposed, 6.4 MB) plus edge index maps; the per-node feature
  table pack = h @ [W | W@al | W@ar] is computed for OWN nodes only and
  AllGathered on-device (NeuronLink), never shipped from host.
- Per layer: mm phase computes pack rows [feat | el | er] for own
  nodes; AllGather -> full table; edge phase: for each 128-edge tile,
  pack[src] rows arrive via indirect DMA gather and er[dst] via a
  second narrow indirect gather; one-hot matrices built on-device from
  dst%128 values (iota + is_equal) turn segment-sum into PE matmuls
  accumulated in PSUM per 128-node block. Softmax denominators are
  divided once per node post-aggregation (exp(e)/sum exp(e) == softmax
  exactly; no max-subtraction needed at these magnitudes).
- Edge-phase element-wise work is batched K tiles at a time with
  broadcast access patterns to cut instruction count.
- The built Bass program and the XLA/NEFF executable are cached across
  kernel() calls (jax persistent compilation cache), so warm calls skip
  the multi-second walrus compile.
"""
import os
import numpy as np
from contextlib import ExitStack

import jax

try:
    jax.config.update("jax_compilation_cache_dir",
                      os.path.expanduser("~/.cache/jax_comp_cache"))
    jax.config.update("jax_persistent_cache_min_compile_time_secs", 0.5)
    jax.config.update("jax_persistent_cache_min_entry_size_bytes", 0)
except Exception:
    pass

import concourse.bass as bass
import concourse.mybir as mybir
import concourse.tile as tile
from concourse.bass_utils import run_bass_kernel_spmd
from concourse.masks import make_identity

try:
    import bass_rust
except ImportError:  # pragma: no cover
    bass_rust = None

F32 = mybir.dt.float32
F16 = mybir.dt.float16
I32 = mybir.dt.int32
ALU = mybir.AluOpType
ACT = mybir.ActivationFunctionType
P = 128
NC = 8
NEG_SLOPE = 0.2
K_GRP = 9

_ws_ctr = [0]


def _split_waits(nc, limit=1):
    """This container's walrus encodes at most `limit` sem waits per
    instruction. Hoist extras onto same-engine NoOps placed before."""
    for fn in nc.m.functions:
        for bb in fn.blocks:
            insts = bb.instructions
            if not any(
                i.sync_info is not None and len(i.sync_info.on_wait) > limit
                for i in insts
            ):
                continue
            out = []
            for ins in insts:
                si = ins.sync_info
                if si is not None and len(si.on_wait) > limit:
                    waits = list(si.on_wait)
                    extra, keep = waits[:-limit], waits[-limit:]
                    for w in extra:
                        _ws_ctr[0] += 1
                        nop = mybir.InstNoOp(
                            name=f"I-waitsplit-{_ws_ctr[0]}", ins=[], outs=[]
                        )
                        nop.engine = ins.engine
                        nop.sync_info = bass_rust.SyncInfo(on_wait=[w], on_update=[])
                        out.append(nop)
                    ins.sync_info = bass_rust.SyncInfo(
                        on_wait=keep, on_update=list(si.on_update)
                    )
                out.append(ins)
            bb.instructions = out


def _pack_cols(n):  # pad row length to a 64-float multiple for clean strides
    return ((n + 63) // 64) * 64


def _build_program(NPAD, T_B, NB):
    NPC = NPAD // NC
    NT = NB * T_B

    nc = bass.Bass(num_devices=NC)

    F_ = (256, 256, 64)             # feature width per layer
    H_ = (4, 4, 1)                  # heads per layer
    C_ = tuple(f + 2 * h for f, h in zip(F_, H_))   # feat + el + er
    E_ = tuple(_pack_cols(c) for c in C_)           # padded pack row

    # ---- DRAM tensors ----
    xTo = nc.dram_tensor("xTo", [256, NPC], F16, kind="ExternalInput")
    idx_h = nc.dram_tensor("idx", [P, NT], I32, kind="ExternalInput")
    dgi_h = nc.dram_tensor("dgi", [P, NT], I32, kind="ExternalInput")
    dstv_h = nc.dram_tensor("dstv", [P, NT], F32, kind="ExternalInput")
    wag = [
        nc.dram_tensor(f"wag{i}", [2, P, c], F32, kind="ExternalInput")
        for i, c in enumerate(C_)
    ]
    rw2 = nc.dram_tensor("rw2", [2, P, 64], F32, kind="ExternalInput")
    bia = [
        nc.dram_tensor(f"bias{i}", [P, f], F32, kind="ExternalInput")
        for i, f in enumerate(F_)
    ]
    out2 = nc.dram_tensor("out2", [NPC, 64], F16, kind="ExternalOutput")

    pko = [
        nc.dram_tensor(f"pko{i}", [NPC, e], F32) for i, e in enumerate(E_)
    ]
    pka = [
        nc.dram_tensor(f"pka{i}", [NPAD, e], F32, addr_space="Shared")
        for i, e in enumerate(E_)
    ]
    hown = nc.dram_tensor("hown", [NPC, 256], F32)
    agin = [nc.dram_tensor(f"agin{i}", [256, NPC], F32) for i in range(2)]

    with tile.TileContext(nc) as tc, ExitStack() as ctx:
        cst = ctx.enter_context(tc.tile_pool(name="cst", bufs=1))
        ld = ctx.enter_context(tc.tile_pool(name="ld", bufs=4))
        stg = ctx.enter_context(tc.tile_pool(name="stg", bufs=4))
        gp = ctx.enter_context(tc.tile_pool(name="gp", bufs=3))
        ep = ctx.enter_context(tc.tile_pool(name="ep", bufs=3))
        sm = ctx.enter_context(tc.tile_pool(name="sm", bufs=4))
        # PSUM: 8 banks/partition total; every tile is padded to one bank.
        mp = ctx.enter_context(tc.tile_pool(name="mp", bufs=2, space="PSUM"))
        m1p = ctx.enter_context(tc.tile_pool(name="m1p", bufs=2, space="PSUM"))
        rp_ = ctx.enter_context(tc.tile_pool(name="rp", bufs=1, space="PSUM"))
        agp = ctx.enter_context(tc.tile_pool(name="agp", bufs=2, space="PSUM"))

        # ---- constants ----
        idx_sb = cst.tile([P, NT], I32, tag="idx")
        nc.sync.dma_start(idx_sb[:], idx_h[:])
        dgi_sb = cst.tile([P, NT], I32, tag="dgi")
        nc.sync.dma_start(dgi_sb[:], dgi_h[:])
        dstv_sb = cst.tile([P, NT], F32, tag="dstv")
        nc.sync.dma_start(dstv_sb[:], dstv_h[:])
        iota_i = cst.tile([P, P], I32, tag="ioi")
        nc.gpsimd.iota(iota_i[:], [[1, P]], channel_multiplier=0)
        iota_f = cst.tile([P, P], F32, tag="iof")
        nc.vector.tensor_copy(iota_f[:], iota_i[:])
        ident = cst.tile([P, P], F32, tag="id")
        make_identity(nc, ident[:])
        wag_sb = []
        for i, c in enumerate(C_):
            t = cst.tile([P, 2, c], F32, tag=f"wag{i}")
            nc.sync.dma_start(t[:, 0, :], wag[i][0])
            nc.sync.dma_start(t[:, 1, :], wag[i][1])
            wag_sb.append(t)
        rw2_sb = cst.tile([P, 2, 64], F32, tag="rw2")
        nc.sync.dma_start(rw2_sb[:, 0, :], rw2[0])
        nc.sync.dma_start(rw2_sb[:, 1, :], rw2[1])
        bia_sb = []
        for i, f in enumerate(F_):
            t = cst.tile([P, f], F32, tag=f"bia{i}")
            nc.sync.dma_start(t[:], bia[i][:])
            bia_sb.append(t)

        def mm_phase(L, h_src):
            """pack rows = h @ [W | W@al | W@ar] for OWN nodes only."""
            C, E = C_[L], E_[L]
            wt = wag_sb[L]
            for b in range(NB):
                c0 = ld.tile([P, P], F32, tag="c0")
                c1 = ld.tile([P, P], F32, tag="c1")
                if L == 0:  # x ships as fp16; cast to f32 after DMA
                    c0h = ld.tile([P, P], F16, tag="c0h")
                    c1h = ld.tile([P, P], F16, tag="c1h")
                    nc.sync.dma_start(c0h[:], h_src[0:P, b * P:(b + 1) * P])
                    nc.sync.dma_start(c1h[:], h_src[P:2 * P, b * P:(b + 1) * P])
                    nc.vector.tensor_copy(c0[:], c0h[:])
                    nc.vector.tensor_copy(c1[:], c1h[:])
                else:
                    nc.sync.dma_start(c0[:], h_src[0:P, b * P:(b + 1) * P])
                    nc.sync.dma_start(c1[:], h_src[P:2 * P, b * P:(b + 1) * P])
                ps = mp.tile([P, max(C_)], F32, tag="mmps", name="mmps",
                             space="PSUM")[:, 0:C]
                nc.tensor.matmul(out=ps[:], lhsT=c0[:], rhs=wt[:, 0, :],
                                 start=True, stop=False)
                nc.tensor.matmul(out=ps[:], lhsT=c1[:], rhs=wt[:, 1, :],
                                 start=False, stop=True)
                st = stg.tile([P, max(E_)], F32, tag="stg")
                nc.vector.tensor_copy(st[:, 0:C], ps[:])
                nc.sync.dma_start(pko[L][b * P:(b + 1) * P, :], st[:, 0:E])

        def edge_phase(L, write_sinks):
            F, H, E = F_[L], H_[L], E_[L]
            pk = pka[L]
            for b in range(NB):
                agf = agp.tile([P, 272], F32, tag="agg", name="agg", space="PSUM")
                agg = agf[:, 0:F]
                den = agf[:, F:F + H]
                for t0 in range(0, T_B, K_GRP):
                    k = min(K_GRP, T_B - t0)
                    tt0 = b * T_B + t0
                    gw = gp.tile([P, K_GRP, E], F32, tag="gw")
                    ge = gp.tile([P, K_GRP, H], F32, tag="ge")
                    m1w = ep.tile([P, K_GRP, P], F32, tag="m1w")
                    for j in range(k):
                        nc.gpsimd.indirect_dma_start(
                            out=gw[:, j, :], out_offset=None, in_=pk[:],
                            in_offset=bass.IndirectOffsetOnAxis(
                                ap=idx_sb[:, tt0 + j:tt0 + j + 1], axis=0))
                        nc.gpsimd.indirect_dma_start(
                            out=ge[:, j, :], out_offset=None, in_=pko[L][:],
                            element_offset=F + H,
                            in_offset=bass.IndirectOffsetOnAxis(
                                ap=dgi_sb[:, tt0 + j:tt0 + j + 1], axis=0))
                    # one-hot of dst%128 for k tiles in one op
                    nc.vector.tensor_tensor(
                        out=m1w[:, 0:k, :],
                        in0=dstv_sb[:, tt0:tt0 + k].unsqueeze(2)
                            .to_broadcast([P, k, P]),
                        in1=iota_f[:].unsqueeze(1).to_broadcast([P, k, P]),
                        op=ALU.is_equal)
                    ew = ep.tile([P, K_GRP, H], F32, tag="ew")
                    nc.vector.tensor_add(ew[:, 0:k, :], gw[:, 0:k, F:F + H],
                                         ge[:, 0:k, :])
                    nc.vector.scalar_tensor_tensor(
                        out=ew[:, 0:k, :], in0=ew[:, 0:k, :], scalar=NEG_SLOPE,
                        in1=ew[:, 0:k, :], op0=ALU.mult, op1=ALU.max)
                    sc = ep.tile([P, K_GRP, F + H], F32, tag="sc")
                    nc.scalar.activation(sc[:, 0:k, F:F + H], ew[:, 0:k, :],
                                         ACT.Exp)
                    nc.vector.tensor_tensor(
                        out=sc[:, 0:k, 0:F].rearrange("p k (h d) -> p k h d", h=H),
                        in0=gw[:, 0:k, 0:F].rearrange("p k (h d) -> p k h d", h=H),
                        in1=sc[:, 0:k, F:F + H].unsqueeze(3)
                            .to_broadcast([P, k, H, F // H]),
                        op=ALU.mult)
                    for j in range(k):
                        nc.tensor.matmul(out=agf[:, 0:F + H], lhsT=m1w[:, j, :],
                                         rhs=sc[:, j, :],
                                         start=(t0 + j == 0),
                                         stop=(t0 + j == T_B - 1))
                # epilogue
                den_c = ep.tile([P, H], F32, tag="denc")
                nc.vector.tensor_scalar_max(den_c[:], den[:], 1e-30)
                rec = ep.tile([P, H], F32, tag="rec")
                nc.vector.reciprocal(rec[:], den_c[:])
                o = ep.tile([P, F], F32, tag="o")
                nc.vector.tensor_tensor(
                    out=o[:].rearrange("p (h d) -> p h d", h=H),
                    in0=agg[:].rearrange("p (h d) -> p h d", h=H),
                    in1=rec[:].to_broadcast([P, H, F // H]), op=ALU.mult)
                write_sinks(b, o)

        def sink_l0(b, o):
            nc.vector.tensor_add(o[:], o[:], bia_sb[0][:])
            _elu(o)
            nc.sync.dma_start(hown[b * P:(b + 1) * P, :], o[:])
            _write_agin(agin[0], b, o)

        def sink_l1(b, o):
            hb = ld.tile([P, 256], F32, tag="hb")
            nc.sync.dma_start(hb[:], hown[b * P:(b + 1) * P, :])
            nc.vector.tensor_add(o[:], o[:], hb[:])
            nc.vector.tensor_add(o[:], o[:], bia_sb[1][:])
            _elu(o)
            _write_agin(agin[1], b, o)

        def sink_l2(b, o):
            r0 = ld.tile([P, P], F32, tag="r0")
            r1 = ld.tile([P, P], F32, tag="r1")
            nc.sync.dma_start(r0[:], agin[1][0:P, b * P:(b + 1) * P])
            nc.sync.dma_start(r1[:], agin[1][P:2 * P, b * P:(b + 1) * P])
            rp = rp_.tile([P, 64], F32, tag="resps", space="PSUM")
            nc.tensor.matmul(out=rp[:], lhsT=r0[:], rhs=rw2_sb[:, 0, :],
                             start=True, stop=False)
            nc.tensor.matmul(out=rp[:], lhsT=r1[:], rhs=rw2_sb[:, 1, :],
                             start=False, stop=True)
            nc.vector.tensor_add(o[:], o[:], rp[:])
            nc.vector.tensor_add(o[:], o[:], bia_sb[2][:])
            of = sm.tile([P, 64], F16, tag="of")
            nc.vector.tensor_copy(of[:], o[:])
            nc.sync.dma_start(out2[b * P:(b + 1) * P, :], of[:])

        def _elu(o):
            mx = sm.tile([P, 256], F32, tag="mx")
            nc.vector.tensor_scalar_max(mx[:], o[:], 0.0)
            mn = sm.tile([P, 256], F32, tag="mn")
            nc.vector.tensor_scalar_min(mn[:], o[:], 0.0)
            exn = sm.tile([P, 256], F32, tag="exn")
            nc.scalar.activation(exn[:], mn[:], ACT.Exp)
            nc.vector.scalar_tensor_tensor(
                out=o[:], in0=exn[:], scalar=-1.0, in1=mx[:],
                op0=ALU.add, op1=ALU.add)

        def _write_agin(ag, b, o):
            t1 = m1p.tile([P, P], F32, tag="m1tps", space="PSUM")
            nc.tensor.transpose(out=t1[:], in_=o[:, 0:P], identity=ident[:])
            ot1 = sm.tile([P, P], F32, tag="ot1")
            nc.vector.tensor_copy(ot1[:], t1[:])
            nc.sync.dma_start(ag[0:P, b * P:(b + 1) * P], ot1[:])
            t2 = m1p.tile([P, P], F32, tag="m1tps", space="PSUM")
            nc.tensor.transpose(out=t2[:], in_=o[:, P:2 * P], identity=ident[:])
            ot2 = sm.tile([P, P], F32, tag="ot2")
            nc.vector.tensor_copy(ot2[:], t2[:])
            nc.sync.dma_start(ag[P:2 * P, b * P:(b + 1) * P], ot2[:])

        def allgather(L):
            tc.strict_bb_all_engine_barrier()
            nc.gpsimd.collective_compute(
                "AllGather", ALU.bypass, replica_groups=[list(range(NC))],
                ins=[pko[L][:]], outs=[pka[L][:]])
            tc.strict_bb_all_engine_barrier()

        # ---- layer 0 ----
        mm_phase(0, xTo)
        allgather(0)
        edge_phase(0, sink_l0)
        tc.strict_bb_all_engine_barrier()
        # ---- layer 1 ----
        mm_phase(1, agin[0])
        allgather(1)
        edge_phase(1, sink_l1)
        tc.strict_bb_all_engine_barrier()
        # ---- layer 2 ----
        mm_phase(2, agin[1])
        allgather(2)
        edge_phase(2, sink_l2)

    _split_waits(nc, limit=1)
    return nc


_PROG_CACHE = {}


def prepare(**inputs):
    x = np.asarray(inputs["x"], dtype=np.float32)
    src = np.asarray(inputs["src"], dtype=np.int64)
    dst = np.asarray(inputs["dst"], dtype=np.int64)
    N, IND = x.shape
    NPAD = ((N + NC * P - 1) // (NC * P)) * (NC * P)
    NPC = NPAD // NC
    NB = NPC // P

    # ---- host-side graph preprocessing (sharding) ----
    core = dst // NPC
    blk = (dst % NPC) // P
    dv = (dst % P).astype(np.float32)
    order = np.lexsort((src, blk, core))
    src_s, dst_s, core_s, blk_s, dv_s = (
        src[order], dst[order], core[order], blk[order], dv[order])
    # per (core, block) counts
    counts = np.zeros((NC, NB), dtype=np.int64)
    np.add.at(counts, (core_s, blk_s), 1)
    T_B = int(np.max((counts + P - 1) // P))
    NT = NB * T_B
    idx_all = np.zeros((NC, NT * P), dtype=np.int32)          # pad idx -> row 0
    dgi_all = np.zeros((NC, NT * P), dtype=np.int32)          # pad dgi -> row 0
    dvv_all = np.full((NC, NT * P), 999.0, dtype=np.float32)  # pad dstv OOR
    for c in range(NC):
        m = core_s == c
        bc = np.concatenate([[0], np.cumsum(counts[c])])
        sc_, dc_, bs_, dvv_ = src_s[m], dst_s[m], blk_s[m], dv_s[m]
        for b in range(NB):
            seg = slice(bc[b], bc[b + 1])
            n = bc[b + 1] - bc[b]
            base = b * T_B * P
            idx_all[c, base:base + n] = sc_[seg]
            dgi_all[c, base:base + n] = dc_[seg] % NPC
            dvv_all[c, base:base + n] = dvv_[seg]
    # wrap position i -> (partition i%128, col i//128)
    idx_maps = idx_all.reshape(NC, NT, P).transpose(0, 2, 1)   # [NC, P, NT]
    dgi_maps = dgi_all.reshape(NC, NT, P).transpose(0, 2, 1)
    dvv_maps = dvv_all.reshape(NC, NT, P).transpose(0, 2, 1)

    # ---- weights prep: wag = [W | W@al | W@ar] ----
    def aug(W, al, ar):
        H, D = al.shape
        alc = np.stack([W[:, h * D:(h + 1) * D] @ al[h] for h in range(H)], axis=1)
        arc = np.stack([W[:, h * D:(h + 1) * D] @ ar[h] for h in range(H)], axis=1)
        return np.concatenate([W, alc, arc], axis=1).astype(np.float32)

    wag0 = aug(inputs["W0"], inputs["al0"], inputs["ar0"])
    wag1 = aug(inputs["W1"], inputs["al1"], inputs["ar1"])
    wag2 = aug(inputs["W2"], inputs["al2"], inputs["ar2"])
    b0 = np.asarray(inputs["b0"], np.float32)
    b1 = np.asarray(inputs["b1"], np.float32)
    b2 = np.asarray(inputs["b2"], np.float32)
    rw2 = np.asarray(inputs["res_w2"], np.float32)

    xpad = np.zeros((NPAD, IND), np.float32)
    xpad[:N] = x

    key = (NPAD, T_B, NB)
    if key not in _PROG_CACHE:
        _PROG_CACHE[key] = _build_program(NPAD, T_B, NB)
    nc = _PROG_CACHE[key]

    def chunks2(W):  # [256, C] -> [2, 128, C]
        return np.stack([W[0:P], W[P:2 * P]]).astype(np.float32)

    in_maps = []
    for c in range(NC):
        in_maps.append({
            "xTo": np.ascontiguousarray(xpad[c * NPC:(c + 1) * NPC].T.astype(np.float16)),
            "idx": np.ascontiguousarray(idx_maps[c]),
            "dgi": np.ascontiguousarray(dgi_maps[c]),
            "dstv": np.ascontiguousarray(dvv_maps[c]),
            "wag0": chunks2(wag0), "wag1": chunks2(wag1), "wag2": chunks2(wag2),
            "rw2": chunks2(rw2),
            "bias0": np.tile(b0[None, :], (P, 1)).astype(np.float32),
            "bias1": np.tile(b1[None, :], (P, 1)).astype(np.float32),
            "bias2": np.tile(b2[None, :], (P, 1)).astype(np.float32),
        })

    return nc, in_maps, N


def kernel(**inputs):
    nc, in_maps, N = prepare(**inputs)
    import time as _time
    _t0 = _time.time()
    res = run_bass_kernel_spmd(nc, in_maps, list(range(NC)))
    global LAST_EXEC_WALL
    LAST_EXEC_WALL = _time.time() - _t0
    out = np.concatenate([res.results[c]["out2"] for c in range(NC)], axis=0)
    return out[:N].astype(np.float32)


# revision 7
# speedup vs baseline: 12.5635x; 1.1695x over previous
"""3-layer GAT on 8 Trainium2 NeuronCores (Bass/Tile, SPMD).

Strategy (dst-sharded edge parallelism, shard-computed pack tables):
- Pad N to NPAD = 8*128*k nodes; core c owns the contiguous node range
  [c*NPC, (c+1)*NPC) and processes exactly the edges whose dst falls in
  its range (edges sorted by dst on host). Weights replicated.
- Host->device traffic is minimized (the axon tunnel runs at ~40 MB/s,
  so it dominates wall time): each core receives ONLY its own slice of
  x (transposed, 6.4 MB) plus edge index maps; the per-node feature
  table pack = h @ [W | W@al | W@ar] is computed for OWN nodes only and
  AllGathered on-device (NeuronLink), never shipped from host.
- Per layer: mm phase computes pack rows [feat | el | er] for own
  nodes; AllGather -> full table; edge phase: for each 128-edge tile,
  pack[src] rows arrive via indirect DMA gather and er[dst] via a
  second narrow indirect gather; one-hot matrices built on-device from
  dst%128 values (iota + is_equal) turn segment-sum into PE matmuls
  accumulated in PSUM per 128-node block. Softmax denominators are
  divided once per node post-aggregation (exp(e)/sum exp(e) == softmax
  exactly; no max-subtraction needed at these magnitudes).
- Edge-phase element-wise work is batched K tiles at a time with
  broadcast access patterns to cut instruction count.
- The built Bass program and the XLA/NEFF executable are cached across
  kernel() calls (jax persistent compilation cache), so warm calls skip
  the multi-second walrus compile.
"""
import os
import numpy as np
from contextlib import ExitStack

import jax

try:
    jax.config.update("jax_compilation_cache_dir",
                      os.path.expanduser("~/.cache/jax_comp_cache"))
    jax.config.update("jax_persistent_cache_min_compile_time_secs", 0.5)
    jax.config.update("jax_persistent_cache_min_entry_size_bytes", 0)
except Exception:
    pass

import concourse.bass as bass
import concourse.mybir as mybir
import concourse.tile as tile
from concourse.bass_utils import run_bass_kernel_spmd
from concourse.masks import make_identity

try:
    import bass_rust
except ImportError:  # pragma: no cover
    bass_rust = None

F32 = mybir.dt.float32
F16 = mybir.dt.float16
I32 = mybir.dt.int32
U16 = mybir.dt.uint16
I16 = mybir.dt.int16
ALU = mybir.AluOpType
ACT = mybir.ActivationFunctionType
P = 128
NC = 8
NEG_SLOPE = 0.2
K_GRP = 9

_ws_ctr = [0]


def _split_waits(nc, limit=1):
    """This container's walrus encodes at most `limit` sem waits per
    instruction. Hoist extras onto same-engine NoOps placed before."""
    for fn in nc.m.functions:
        for bb in fn.blocks:
            insts = bb.instructions
            if not any(
                i.sync_info is not None and len(i.sync_info.on_wait) > limit
                for i in insts
            ):
                continue
            out = []
            for ins in insts:
                si = ins.sync_info
                if si is not None and len(si.on_wait) > limit:
                    waits = list(si.on_wait)
                    extra, keep = waits[:-limit], waits[-limit:]
                    for w in extra:
                        _ws_ctr[0] += 1
                        nop = mybir.InstNoOp(
                            name=f"I-waitsplit-{_ws_ctr[0]}", ins=[], outs=[]
                        )
                        nop.engine = ins.engine
                        nop.sync_info = bass_rust.SyncInfo(on_wait=[w], on_update=[])
                        out.append(nop)
                    ins.sync_info = bass_rust.SyncInfo(
                        on_wait=keep, on_update=list(si.on_update)
                    )
                out.append(ins)
            bb.instructions = out


def _pack_cols(n):  # pad row length to a 64-float multiple for clean strides
    return ((n + 63) // 64) * 64


def _build_program(NPAD, T_B, NB):
    NPC = NPAD // NC
    NT = NB * T_B

    nc = bass.Bass(num_devices=NC)

    F_ = (256, 256, 64)             # feature width per layer
    H_ = (4, 4, 1)                  # heads per layer
    C_ = tuple(f + 2 * h for f, h in zip(F_, H_))   # feat + el + er
    E_ = tuple(_pack_cols(c) for c in C_)           # padded pack row

    # ---- DRAM tensors ----
    xTo = nc.dram_tensor("xTo", [256, NPC], F16, kind="ExternalInput")
    idx_h = nc.dram_tensor("idx", [P, NT], U16, kind="ExternalInput")
    dgi_h = nc.dram_tensor("dgi", [P, NT], U16, kind="ExternalInput")
    dstv_h = nc.dram_tensor("dstv", [P, NT], I16, kind="ExternalInput")
    wag = [
        nc.dram_tensor(f"wag{i}", [2, P, c], F32, kind="ExternalInput")
        for i, c in enumerate(C_)
    ]
    rw2 = nc.dram_tensor("rw2", [2, P, 64], F32, kind="ExternalInput")
    bia = [
        nc.dram_tensor(f"bias{i}", [P, f], F32, kind="ExternalInput")
        for i, f in enumerate(F_)
    ]
    out2 = nc.dram_tensor("out2", [NPC, 64], F16, kind="ExternalOutput")

    pko = [
        nc.dram_tensor(f"pko{i}", [NPC, e], F32) for i, e in enumerate(E_)
    ]
    pka = [
        nc.dram_tensor(f"pka{i}", [NPAD, e], F32, addr_space="Shared")
        for i, e in enumerate(E_)
    ]
    hown = nc.dram_tensor("hown", [NPC, 256], F32)
    agin = [nc.dram_tensor(f"agin{i}", [256, NPC], F32) for i in range(2)]

    with tile.TileContext(nc) as tc, ExitStack() as ctx:
        cst = ctx.enter_context(tc.tile_pool(name="cst", bufs=1))
        ld = ctx.enter_context(tc.tile_pool(name="ld", bufs=4))
        stg = ctx.enter_context(tc.tile_pool(name="stg", bufs=4))
        gp = ctx.enter_context(tc.tile_pool(name="gp", bufs=3))
        ep = ctx.enter_context(tc.tile_pool(name="ep", bufs=3))
        sm = ctx.enter_context(tc.tile_pool(name="sm", bufs=4))
        # PSUM: 8 banks/partition total; every tile is padded to one bank.
        mp = ctx.enter_context(tc.tile_pool(name="mp", bufs=2, space="PSUM"))
        m1p = ctx.enter_context(tc.tile_pool(name="m1p", bufs=2, space="PSUM"))
        rp_ = ctx.enter_context(tc.tile_pool(name="rp", bufs=1, space="PSUM"))
        agp = ctx.enter_context(tc.tile_pool(name="agp", bufs=2, space="PSUM"))

        # ---- constants ----
        idx_raw = ld.tile([P, NT], U16, tag="idxr")
        nc.sync.dma_start(idx_raw[:], idx_h[:])
        idx_sb = cst.tile([P, NT], I32, tag="idx")
        nc.vector.tensor_copy(idx_sb[:], idx_raw[:])
        dgi_raw = ld.tile([P, NT], U16, tag="dgir")
        nc.sync.dma_start(dgi_raw[:], dgi_h[:])
        dgi_sb = cst.tile([P, NT], I32, tag="dgi")
        nc.vector.tensor_copy(dgi_sb[:], dgi_raw[:])
        dstv_raw = ld.tile([P, NT], I16, tag="dstr")
        nc.sync.dma_start(dstv_raw[:], dstv_h[:])
        dstv_sb = cst.tile([P, NT], F32, tag="dstv")
        nc.vector.tensor_copy(dstv_sb[:], dstv_raw[:])
        iota_i = cst.tile([P, P], I32, tag="ioi")
        nc.gpsimd.iota(iota_i[:], [[1, P]], channel_multiplier=0)
        iota_f = cst.tile([P, P], F32, tag="iof")
        nc.vector.tensor_copy(iota_f[:], iota_i[:])
        ident = cst.tile([P, P], F32, tag="id")
        make_identity(nc, ident[:])
        wag_sb = []
        for i, c in enumerate(C_):
            t = cst.tile([P, 2, c], F32, tag=f"wag{i}")
            nc.sync.dma_start(t[:, 0, :], wag[i][0])
            nc.sync.dma_start(t[:, 1, :], wag[i][1])
            wag_sb.append(t)
        rw2_sb = cst.tile([P, 2, 64], F32, tag="rw2")
        nc.sync.dma_start(rw2_sb[:, 0, :], rw2[0])
        nc.sync.dma_start(rw2_sb[:, 1, :], rw2[1])
        bia_sb = []
        for i, f in enumerate(F_):
            t = cst.tile([P, f], F32, tag=f"bia{i}")
            nc.sync.dma_start(t[:], bia[i][:])
            bia_sb.append(t)

        def mm_phase(L, h_src):
            """pack rows = h @ [W | W@al | W@ar] for OWN nodes only."""
            C, E = C_[L], E_[L]
            wt = wag_sb[L]
            for b in range(NB):
                c0 = ld.tile([P, P], F32, tag="c0")
                c1 = ld.tile([P, P], F32, tag="c1")
                if L == 0:  # x ships as fp16; cast to f32 after DMA
                    c0h = ld.tile([P, P], F16, tag="c0h")
                    c1h = ld.tile([P, P], F16, tag="c1h")
                    nc.sync.dma_start(c0h[:], h_src[0:P, b * P:(b + 1) * P])
                    nc.sync.dma_start(c1h[:], h_src[P:2 * P, b * P:(b + 1) * P])
                    nc.vector.tensor_copy(c0[:], c0h[:])
                    nc.vector.tensor_copy(c1[:], c1h[:])
                else:
                    nc.sync.dma_start(c0[:], h_src[0:P, b * P:(b + 1) * P])
                    nc.sync.dma_start(c1[:], h_src[P:2 * P, b * P:(b + 1) * P])
                ps = mp.tile([P, max(C_)], F32, tag="mmps", name="mmps",
                             space="PSUM")[:, 0:C]
                nc.tensor.matmul(out=ps[:], lhsT=c0[:], rhs=wt[:, 0, :],
                                 start=True, stop=False)
                nc.tensor.matmul(out=ps[:], lhsT=c1[:], rhs=wt[:, 1, :],
                                 start=False, stop=True)
                st = stg.tile([P, max(E_)], F32, tag="stg")
                nc.vector.tensor_copy(st[:, 0:C], ps[:])
                nc.sync.dma_start(pko[L][b * P:(b + 1) * P, :], st[:, 0:E])

        def edge_phase(L, write_sinks):
            F, H, E = F_[L], H_[L], E_[L]
            pk = pka[L]
            for b in range(NB):
                agf = agp.tile([P, 272], F32, tag="agg", name="agg", space="PSUM")
                agg = agf[:, 0:F]
                den = agf[:, F:F + H]
                for t0 in range(0, T_B, K_GRP):
                    k = min(K_GRP, T_B - t0)
                    tt0 = b * T_B + t0
                    gw = gp.tile([P, K_GRP, E], F32, tag="gw")
                    ge = gp.tile([P, K_GRP, H], F32, tag="ge")
                    m1w = ep.tile([P, K_GRP, P], F32, tag="m1w")
                    for j in range(k):
                        nc.gpsimd.indirect_dma_start(
                            out=gw[:, j, :], out_offset=None, in_=pk[:],
                            in_offset=bass.IndirectOffsetOnAxis(
                                ap=idx_sb[:, tt0 + j:tt0 + j + 1], axis=0))
                        nc.gpsimd.indirect_dma_start(
                            out=ge[:, j, :], out_offset=None, in_=pko[L][:],
                            element_offset=F + H,
                            in_offset=bass.IndirectOffsetOnAxis(
                                ap=dgi_sb[:, tt0 + j:tt0 + j + 1], axis=0))
                    # one-hot of dst%128 for k tiles in one op
                    nc.vector.tensor_tensor(
                        out=m1w[:, 0:k, :],
                        in0=dstv_sb[:, tt0:tt0 + k].unsqueeze(2)
                            .to_broadcast([P, k, P]),
                        in1=iota_f[:].unsqueeze(1).to_broadcast([P, k, P]),
                        op=ALU.is_equal)
                    ew = ep.tile([P, K_GRP, H], F32, tag="ew")
                    nc.vector.tensor_add(ew[:, 0:k, :], gw[:, 0:k, F:F + H],
                                         ge[:, 0:k, :])
                    nc.vector.scalar_tensor_tensor(
                        out=ew[:, 0:k, :], in0=ew[:, 0:k, :], scalar=NEG_SLOPE,
                        in1=ew[:, 0:k, :], op0=ALU.mult, op1=ALU.max)
                    sc = ep.tile([P, K_GRP, F + H], F32, tag="sc")
                    nc.scalar.activation(sc[:, 0:k, F:F + H], ew[:, 0:k, :],
                                         ACT.Exp)
                    nc.vector.tensor_tensor(
                        out=sc[:, 0:k, 0:F].rearrange("p k (h d) -> p k h d", h=H),
                        in0=gw[:, 0:k, 0:F].rearrange("p k (h d) -> p k h d", h=H),
                        in1=sc[:, 0:k, F:F + H].unsqueeze(3)
                            .to_broadcast([P, k, H, F // H]),
                        op=ALU.mult)
                    for j in range(k):
                        nc.tensor.matmul(out=agf[:, 0:F + H], lhsT=m1w[:, j, :],
                                         rhs=sc[:, j, :],
                                         start=(t0 + j == 0),
                                         stop=(t0 + j == T_B - 1))
                # epilogue
                den_c = ep.tile([P, H], F32, tag="denc")
                nc.vector.tensor_scalar_max(den_c[:], den[:], 1e-30)
                rec = ep.tile([P, H], F32, tag="rec")
                nc.vector.reciprocal(rec[:], den_c[:])
                o = ep.tile([P, F], F32, tag="o")
                nc.vector.tensor_tensor(
                    out=o[:].rearrange("p (h d) -> p h d", h=H),
                    in0=agg[:].rearrange("p (h d) -> p h d", h=H),
                    in1=rec[:].to_broadcast([P, H, F // H]), op=ALU.mult)
                write_sinks(b, o)

        def sink_l0(b, o):
            nc.vector.tensor_add(o[:], o[:], bia_sb[0][:])
            _elu(o)
            nc.sync.dma_start(hown[b * P:(b + 1) * P, :], o[:])
            _write_agin(agin[0], b, o)

        def sink_l1(b, o):
            hb = ld.tile([P, 256], F32, tag="hb")
            nc.sync.dma_start(hb[:], hown[b * P:(b + 1) * P, :])
            nc.vector.tensor_add(o[:], o[:], hb[:])
            nc.vector.tensor_add(o[:], o[:], bia_sb[1][:])
            _elu(o)
            _write_agin(agin[1], b, o)

        def sink_l2(b, o):
            r0 = ld.tile([P, P], F32, tag="r0")
            r1 = ld.tile([P, P], F32, tag="r1")
            nc.sync.dma_start(r0[:], agin[1][0:P, b * P:(b + 1) * P])
            nc.sync.dma_start(r1[:], agin[1][P:2 * P, b * P:(b + 1) * P])
            rp = rp_.tile([P, 64], F32, tag="resps", space="PSUM")
            nc.tensor.matmul(out=rp[:], lhsT=r0[:], rhs=rw2_sb[:, 0, :],
                             start=True, stop=False)
            nc.tensor.matmul(out=rp[:], lhsT=r1[:], rhs=rw2_sb[:, 1, :],
                             start=False, stop=True)
            nc.vector.tensor_add(o[:], o[:], rp[:])
            nc.vector.tensor_add(o[:], o[:], bia_sb[2][:])
            of = sm.tile([P, 64], F16, tag="of")
            nc.vector.tensor_copy(of[:], o[:])
            nc.sync.dma_start(out2[b * P:(b + 1) * P, :], of[:])

        def _elu(o):
            mx = sm.tile([P, 256], F32, tag="mx")
            nc.vector.tensor_scalar_max(mx[:], o[:], 0.0)
            mn = sm.tile([P, 256], F32, tag="mn")
            nc.vector.tensor_scalar_min(mn[:], o[:], 0.0)
            exn = sm.tile([P, 256], F32, tag="exn")
            nc.scalar.activation(exn[:], mn[:], ACT.Exp)
            nc.vector.scalar_tensor_tensor(
                out=o[:], in0=exn[:], scalar=-1.0, in1=mx[:],
                op0=ALU.add, op1=ALU.add)

        def _write_agin(ag, b, o):
            t1 = m1p.tile([P, P], F32, tag="m1tps", space="PSUM")
            nc.tensor.transpose(out=t1[:], in_=o[:, 0:P], identity=ident[:])
            ot1 = sm.tile([P, P], F32, tag="ot1")
            nc.vector.tensor_copy(ot1[:], t1[:])
            nc.sync.dma_start(ag[0:P, b * P:(b + 1) * P], ot1[:])
            t2 = m1p.tile([P, P], F32, tag="m1tps", space="PSUM")
            nc.tensor.transpose(out=t2[:], in_=o[:, P:2 * P], identity=ident[:])
            ot2 = sm.tile([P, P], F32, tag="ot2")
            nc.vector.tensor_copy(ot2[:], t2[:])
            nc.sync.dma_start(ag[P:2 * P, b * P:(b + 1) * P], ot2[:])

        def allgather(L):
            tc.strict_bb_all_engine_barrier()
            nc.gpsimd.collective_compute(
                "AllGather", ALU.bypass, replica_groups=[list(range(NC))],
                ins=[pko[L][:]], outs=[pka[L][:]])
            tc.strict_bb_all_engine_barrier()

        # ---- layer 0 ----
        mm_phase(0, xTo)
        allgather(0)
        edge_phase(0, sink_l0)
        tc.strict_bb_all_engine_barrier()
        # ---- layer 1 ----
        mm_phase(1, agin[0])
        allgather(1)
        edge_phase(1, sink_l1)
        tc.strict_bb_all_engine_barrier()
        # ---- layer 2 ----
        mm_phase(2, agin[1])
        allgather(2)
        edge_phase(2, sink_l2)

    _split_waits(nc, limit=1)
    return nc


_PROG_CACHE = {}


def prepare(**inputs):
    x = np.asarray(inputs["x"], dtype=np.float32)
    src = np.asarray(inputs["src"], dtype=np.int64)
    dst = np.asarray(inputs["dst"], dtype=np.int64)
    N, IND = x.shape
    NPAD = ((N + NC * P - 1) // (NC * P)) * (NC * P)
    NPC = NPAD // NC
    NB = NPC // P

    # ---- host-side graph preprocessing (sharding) ----
    core = dst // NPC
    blk = (dst % NPC) // P
    dv = (dst % P).astype(np.int16)
    order = np.lexsort((src, blk, core))
    src_s, dst_s, core_s, blk_s, dv_s = (
        src[order], dst[order], core[order], blk[order], dv[order])
    # per (core, block) counts
    counts = np.zeros((NC, NB), dtype=np.int64)
    np.add.at(counts, (core_s, blk_s), 1)
    T_B = int(np.max((counts + P - 1) // P))
    NT = NB * T_B
    idx_all = np.zeros((NC, NT * P), dtype=np.uint16)         # pad idx -> row 0
    dgi_all = np.zeros((NC, NT * P), dtype=np.uint16)         # pad dgi -> row 0
    dvv_all = np.full((NC, NT * P), 999, dtype=np.int16)      # pad dstv OOR
    for c in range(NC):
        m = core_s == c
        bc = np.concatenate([[0], np.cumsum(counts[c])])
        sc_, dc_, bs_, dvv_ = src_s[m], dst_s[m], blk_s[m], dv_s[m]
        for b in range(NB):
            seg = slice(bc[b], bc[b + 1])
            n = bc[b + 1] - bc[b]
            base = b * T_B * P
            idx_all[c, base:base + n] = sc_[seg]
            dgi_all[c, base:base + n] = dc_[seg] % NPC
            dvv_all[c, base:base + n] = dvv_[seg]
    # wrap position i -> (partition i%128, col i//128)
    idx_maps = idx_all.reshape(NC, NT, P).transpose(0, 2, 1)   # [NC, P, NT]
    dgi_maps = dgi_all.reshape(NC, NT, P).transpose(0, 2, 1)
    dvv_maps = dvv_all.reshape(NC, NT, P).transpose(0, 2, 1)

    # ---- weights prep: wag = [W | W@al | W@ar] ----
    def aug(W, al, ar):
        H, D = al.shape
        alc = np.stack([W[:, h * D:(h + 1) * D] @ al[h] for h in range(H)], axis=1)
        arc = np.stack([W[:, h * D:(h + 1) * D] @ ar[h] for h in range(H)], axis=1)
        return np.concatenate([W, alc, arc], axis=1).astype(np.float32)

    wag0 = aug(inputs["W0"], inputs["al0"], inputs["ar0"])
    wag1 = aug(inputs["W1"], inputs["al1"], inputs["ar1"])
    wag2 = aug(inputs["W2"], inputs["al2"], inputs["ar2"])
    b0 = np.asarray(inputs["b0"], np.float32)
    b1 = np.asarray(inputs["b1"], np.float32)
    b2 = np.asarray(inputs["b2"], np.float32)
    rw2 = np.asarray(inputs["res_w2"], np.float32)

    xpad = np.zeros((NPAD, IND), np.float32)
    xpad[:N] = x

    key = (NPAD, T_B, NB)
    if key not in _PROG_CACHE:
        _PROG_CACHE[key] = _build_program(NPAD, T_B, NB)
    nc = _PROG_CACHE[key]

    def chunks2(W):  # [256, C] -> [2, 128, C]
        return np.stack([W[0:P], W[P:2 * P]]).astype(np.float32)

    in_maps = []
    for c in range(NC):
        in_maps.append({
            "xTo": np.ascontiguousarray(xpad[c * NPC:(c + 1) * NPC].T.astype(np.float16)),
            "idx": np.ascontiguousarray(idx_maps[c]),
            "dgi": np.ascontiguousarray(dgi_maps[c]),
            "dstv": np.ascontiguousarray(dvv_maps[c]),
            "wag0": chunks2(wag0), "wag1": chunks2(wag1), "wag2": chunks2(wag2),
            "rw2": chunks2(rw2),
            "bias0": np.tile(b0[None, :], (P, 1)).astype(np.float32),
            "bias1": np.tile(b1[None, :], (P, 1)).astype(np.float32),
            "bias2": np.tile(b2[None, :], (P, 1)).astype(np.float32),
        })

    return nc, in_maps, N


def kernel(**inputs):
    nc, in_maps, N = prepare(**inputs)
    import time as _time
    _t0 = _time.time()
    res = run_bass_kernel_spmd(nc, in_maps, list(range(NC)))
    global LAST_EXEC_WALL
    LAST_EXEC_WALL = _time.time() - _t0
    out = np.concatenate([res.results[c]["out2"] for c in range(NC)], axis=0)
    return out[:N].astype(np.float32)
